# revision 3
# baseline (speedup 1.0000x reference)
"""Conv2d-via-FFT reference implemented as a direct convolution on TRN2.

The reference pads to FFT size 61 >= 32+3-1, so its circular cross-correlation
equals the linear valid cross-correlation: out[n,f,i,j] =
sum_{c,p,q} x[n,c,i+p,j+q] * w[f,c,p,q] + bias[f].  That is an ordinary
stride-1 valid conv2d: 9 accumulated matmuls (one per filter tap) with C=128
on the contraction partitions, fp32 PSUM accumulation.

Sharding: data-parallel over N (64 samples -> 8 per core), filter replicated.

v3: fp16 operands + amortized weight loads.  The fp32r profile showed a
294ns steady cadence = 450-cycle stream + 256-cycle LDWEIGHTS per tap: 36%
of PE time reloading the same stationary tile (fp32r matmuls must
self-load).  fp16 supports standalone ldweights, so each (group, tap)
loads the stationary once and streams 4 samples' chunks through it
(matmuls carry ldweights=False).  fp16 quantization of x/w costs 3.0e-4
rel err (vs 1.3e-4 fp32r) and halves the x DMA bytes.

Groups of 4 PSUM banks ping-pong (samples 0-3 <-> 4-7), so the scalar
engine drains one half while the PE fills the other.  Per-bank tap-8
matmuls bump s_mm individually, so the drain overlaps the fill bank by
bank.
"""

import numpy as np

import concourse.bass as bass
import concourse.bacc as bacc
import concourse.mybir as mybir
from concourse.bass_utils import run_bass_kernel_spmd

dt = mybir.dt
F32 = dt.float32
F16 = dt.float16
IDENT = mybir.ActivationFunctionType.Identity

N, C, H, W = 64, 128, 32, 32
F, KH, KW = 128, 3, 3
KK = KH * KW
OH, OW = H - KH + 1, W - KW + 1          # 30, 30
NCORES = 8
NPC = N // NCORES                        # samples per core
NWARM = 3                                # HAM warmup matmuls

# Groups: (sample0, half) -> 4 PSUM banks, ping-pong banks 0-3 / 4-7.
GROUPS = [(0, 0), (4, 0), (0, 1), (4, 1)]


def _build():
    nc = bacc.Bacc("TRN2", target_bir_lowering=False, debug=False)

    x_d = nc.dram_tensor("x", [C, NPC, H, W], F16, kind="ExternalInput").ap()
    w_d = nc.dram_tensor("w", [C, KK, F], F16, kind="ExternalInput").ap()
    b_d = nc.dram_tensor("bias", [F, 1], F32, kind="ExternalInput").ap()
    o_d = nc.dram_tensor("out", [NPC, F, OH * OW], F32, kind="ExternalOutput").ap()

    w_sb = nc.alloc_sbuf_tensor("w_sb", [C, KK, F], F16).ap()
    b_sb = nc.alloc_sbuf_tensor("b_sb", [F, 1], F32).ap()
    x_sb = nc.alloc_sbuf_tensor("x_sb", [C, NPC, H, W], F16).ap()
    o_sb = nc.alloc_sbuf_tensor("o_sb", [F, 8, 15 * OW], F32).ap()
    # One PSUM tensor spanning all 8 banks: bank j = 512-float column j.
    ps = nc.alloc_psum_tensor("ps", [F, 8, 512], F32).ap()

    # HWDGE semantics: a DMA's +16 arrives as 16 independent +1s (one per
    # SDMA engine), so a sem with more than one DMA in flight may only be
    # waited at its final value.  Every sem here takes exactly one DMA
    # (waited at 16) or engine increments.  Sem numbers pinned at 207+ so
    # the NEFF epilogue's blanket per-engine sem reset stays sound without
    # an exit barrier.
    from contextlib import ExitStack
    with ExitStack() as ctx:
      _next_num = iter(range(207, 255))
      sem = lambda nm: ctx.enter_context(nc.semaphore(nm, num=next(_next_num)))
      s_wg = [sem(f"s_wg{g}") for g in range(3)]      # w tap groups of 3
      s_x0 = [sem(f"s_x0{n}") for n in range(4)]      # G0 per-sample rows 0-16
      s_xg = [None] + [sem(f"s_xg{g}") for g in (1, 2, 3)]  # G1-G3 batched
      s_b = sem("s_b")
      s_o = [sem(f"s_o{j}") for j in range(8)]        # out DMA per o_sb column
      s_mm = sem("s_mm")
      s_act = sem("s_act")

      _orig_barrier = nc.all_engine_barrier
      nc.all_engine_barrier = lambda *a, **k: None
      with nc.Block(no_gpsimd_drain=True) as block:

        @block.sync
        def _(sync):
            # w group 0 ahead of everything (first LDW dependency), then x in
            # the order the tensor engine consumes.  G0 lands per-sample so
            # the stream can start after one sample's 0.28 KB/partition.
            sync.dma_start(w_sb[:, 0:3], w_d[:, 0:3]).then_inc(s_wg[0], 16)
            for n in range(4):
                sync.dma_start(x_sb[:, n, 0:17],
                               x_d[:, n, 0:17]).then_inc(s_x0[n], 16)
            sync.dma_start(x_sb[:, 4:8, 0:17],
                           x_d[:, 4:8, 0:17]).then_inc(s_xg[1], 16)
            sync.dma_start(x_sb[:, 0:4, 17:32],
                           x_d[:, 0:4, 17:32]).then_inc(s_xg[2], 16)
            sync.dma_start(x_sb[:, 4:8, 17:32],
                           x_d[:, 4:8, 17:32]).then_inc(s_xg[3], 16)
            for j in range(8):                        # all outputs in DRAM
                sync.wait_ge(s_o[j], 32)

        @block.scalar
        def _(scalar):
            scalar.dma_start(b_sb[:], b_d[:]).then_inc(s_b, 16)
            scalar.dma_start(w_sb[:, 3:6], w_d[:, 3:6]).then_inc(s_wg[1], 16)
            scalar.dma_start(w_sb[:, 6:9], w_d[:, 6:9]).then_inc(s_wg[2], 16)
            for g, (n0, half) in enumerate(GROUPS):
                for i in range(4):
                    j = 4 * (g % 2) + i               # bank & o_sb column
                    n = n0 + i                        # sample
                    scalar.wait_ge(s_mm, 4 * g + i + 1)   # bank accumulated
                    if g == 0 and i == 0:
                        scalar.wait_ge(s_b, 16)       # bias landed
                    if g >= 2:
                        # o_sb column free once its half-0 out DMA drained
                        scalar.wait_ge(s_o[j], 16)
                    nc.scalar.activation(o_sb[:, j], ps[:, j, :15 * OW],
                                         IDENT, bias=b_sb[:]).then_inc(s_act, 1)
                    scalar.dma_start(
                        o_d[n, :, 450 * half:450 * half + 450],
                        o_sb[:, j]).then_inc(s_o[j], 16)

        @block.tensor
        def _(tensor):
            # Self-loading warmup matmuls on whatever is in SBUF: the PE is
            # busy from kernel entry while the first x DMA lands.  Results
            # go to bank 7, which group 1's start=True matmul resets before
            # any scalar read.
            for _ in range(NWARM):
                nc.tensor.matmul(ps[:, 7, :], w_sb[:, 0], x_sb[:, 0, 0:16, :],
                                 start=True, stop=True, skip_group_check=True)
            for g, (n0, half) in enumerate(GROUPS):
                bank0 = 4 * (g % 2)
                for k in range(KK):
                    p, q = divmod(k, KW)
                    r0 = 15 * half + p
                    ldw = nc.tensor.ldweights(w_sb[:, k])
                    if g == 0 and k in (0, 3, 6):
                        ldw._wait_ge(s_wg[k // 3], 16)   # tap group landed
                    if g >= 1 and k == 0:
                        ldw._wait_ge(s_xg[g], 16)        # x group landed
                    for i in range(4):
                        jb = bank0 + i
                        mm = nc.tensor.matmul(
                            ps[:, jb, :450],
                            w_sb[:, k],
                            x_sb[:, n0 + i, r0:r0 + 15, q:q + OW],
                            start=(k == 0),
                            stop=(k == KK - 1),
                        )
                        mm.ins.ldweights = False
                        if g == 0 and k == 0:
                            mm._wait_ge(s_x0[i], 16)     # sample landed
                        if g >= 2 and k == 0:
                            # bank drained by scalar before we reset it
                            mm._wait_ge(s_act, 4 * (g - 2) + i + 1)
                        if k == KK - 1:
                            mm.then_inc(s_mm, 1)

      nc.all_engine_barrier = _orig_barrier

    nc.compile()
    return nc


_NC = None


def _get_nc():
    global _NC
    if _NC is None:
        _NC = _build()
    return _NC


def _in_maps(x, w, bias):
    w_prep = np.ascontiguousarray(
        w.transpose(1, 2, 3, 0).reshape(C, KK, F).astype(np.float16))
    b_prep = np.ascontiguousarray(bias.astype(np.float32).reshape(F, 1))
    maps = []
    for c in range(NCORES):
        xc = np.ascontiguousarray(
            x[c * NPC:(c + 1) * NPC].transpose(1, 0, 2, 3).astype(np.float16))
        maps.append({"x": xc, "w": w_prep, "bias": b_prep})
    return maps


def run(x, w, bias, trace=False, **spmd_kwargs):
    """Run the SPMD kernel; returns (out [N,F,OH,OW], BassKernelResults)."""
    nc = _get_nc()
    res = run_bass_kernel_spmd(nc, _in_maps(x, w, bias), list(range(NCORES)),
                               trace=trace, **spmd_kwargs)
    parts = [res.results[c]["out"].reshape(NPC, F, OH, OW) for c in range(NCORES)]
    return np.concatenate(parts, axis=0), res


def kernel(x, w, bias):
    out, _ = run(np.asarray(x), np.asarray(w), np.asarray(bias))
    return out


# revision 7
# speedup vs baseline: 1.1199x; 1.1199x over previous
"""Conv2d-via-FFT reference implemented as a direct convolution on TRN2.

The reference pads to FFT size 61 >= 32+3-1, so its circular cross-correlation
equals the linear valid cross-correlation: out[n,f,i,j] =
sum_{c,p,q} x[n,c,i+p,j+q] * w[f,c,p,q] + bias[f].  That is an ordinary
stride-1 valid conv2d: 9 accumulated matmuls (one per filter tap) with C=128
on the contraction partitions, fp32 PSUM accumulation.

Sharding: data-parallel over N (64 samples -> 8 per core), filter replicated.

v4: fp16 operands, amortized weight loads, two-engine drain.
 - fp16 supports standalone ldweights, so each (group, tap) loads the
   stationary once and streams 4 samples' chunks through it (matmuls carry
   ldweights=False): 36 LDWEIGHTS instead of 144.  fp16 quantization costs
   3.0e-4 rel err and halves the x DMA bytes.
 - Groups of 4 PSUM banks ping-pong (samples 0-3 <-> 4-7); scalar drains
   even banks, vector odd banks (ACTIVATE / tensor_scalar_add with the
   bias), each issuing its own out DMAs, so the post-stream drain tail is
   one 450-element activation + one store instead of four.
 - The final bank is split in half across both engines to shorten the
   last ACT+DMA chain.
 - 10 warmup matmuls keep the PE busy from kernel entry so the HAM clock
   gate (3us of continuous execution) opens before the real stream starts;
   output completeness is guaranteed by the scalar/vector block-exit
   DRAINs, so no engine sits on final semaphore waits.
"""

import numpy as np

import concourse.bass as bass
import concourse.bacc as bacc
import concourse.mybir as mybir
from concourse.bass_utils import run_bass_kernel_spmd

dt = mybir.dt
F32 = dt.float32
F16 = dt.float16
IDENT = mybir.ActivationFunctionType.Identity

N, C, H, W = 64, 128, 32, 32
F, KH, KW = 128, 3, 3
KK = KH * KW
OH, OW = H - KH + 1, W - KW + 1          # 30, 30
NCORES = 8
NPC = N // NCORES                        # samples per core
NWARM = 10                               # HAM warmup matmuls
PX = 15 * OW                             # 450 columns per bank

# Groups: (sample0, half) -> 4 PSUM banks, ping-pong banks 0-3 / 4-7.
GROUPS = [(0, 0), (4, 0), (0, 1), (4, 1)]


def _build():
    nc = bacc.Bacc("TRN2", target_bir_lowering=False, debug=False)

    x_d = nc.dram_tensor("x", [C, NPC, H, W], F16, kind="ExternalInput").ap()
    w_d = nc.dram_tensor("w", [C, KK, F], F16, kind="ExternalInput").ap()
    b_d = nc.dram_tensor("bias", [F, 1], F32, kind="ExternalInput").ap()
    o_d = nc.dram_tensor("out", [NPC, F, OH * OW], F32, kind="ExternalOutput").ap()

    w_sb = nc.alloc_sbuf_tensor("w_sb", [C, KK, F], F16).ap()
    b_sb = nc.alloc_sbuf_tensor("b_sb", [F, 1], F32).ap()
    x_sb = nc.alloc_sbuf_tensor("x_sb", [C, NPC, H, W], F16).ap()
    o_sb = nc.alloc_sbuf_tensor("o_sb", [F, 8, PX], F32).ap()
    # One PSUM tensor spanning all 8 banks: bank j = 512-float column j.
    ps = nc.alloc_psum_tensor("ps", [F, 8, 512], F32).ap()

    from contextlib import ExitStack
    with ExitStack() as ctx:
      _next_num = iter(range(207, 255))
      sem = lambda nm: ctx.enter_context(nc.semaphore(nm, num=next(_next_num)))
      s_wg = [sem(f"s_wg{g}") for g in range(3)]      # w tap groups of 3
      s_x0 = [sem(f"s_x0{n}") for n in range(4)]      # G0 per-sample rows 0-16
      s_xg = [None] + [sem(f"s_xg{g}") for g in (1, 2, 3)]  # G1-G3 batched
      s_b = sem("s_b")
      s_o = [sem(f"s_o{j}") for j in range(8)]        # out DMA per o_sb column
      s_mm = sem("s_mm")
      s_act_s = sem("s_act_s")                        # scalar-drained banks
      s_act_v = sem("s_act_v")                        # vector-drained banks

      _orig_barrier = nc.all_engine_barrier
      nc.all_engine_barrier = lambda *a, **k: None
      with nc.Block(no_gpsimd_drain=True) as block:

        # Vector's drained banks, in its processing order (cnt = s_act_v).
        VBANKS = []
        for g, (n0, half) in enumerate(GROUPS):
            for i in (1, 3):
                j = 4 * (g % 2) + i
                last = (g == 3 and i == 3)
                VBANKS.append((j, n0 + i, half, 0, PX // 2 if last else PX))

        @block.sync
        def _(sync):
            # w group 0 ahead of everything (first LDW dependency), then x in
            # the order the tensor engine consumes.  G0 lands per-sample so
            # the stream can start after one sample's 1.1 KB/partition.
            sync.dma_start(w_sb[:, 0:3], w_d[:, 0:3]).then_inc(s_wg[0], 16)
            for n in range(4):
                sync.dma_start(x_sb[:, n, 0:17],
                               x_d[:, n, 0:17]).then_inc(s_x0[n], 16)
            sync.dma_start(x_sb[:, 4:8, 0:17],
                           x_d[:, 4:8, 0:17]).then_inc(s_xg[1], 16)
            sync.dma_start(x_sb[:, 0:4, 17:32],
                           x_d[:, 0:4, 17:32]).then_inc(s_xg[2], 16)
            sync.dma_start(x_sb[:, 4:8, 17:32],
                           x_d[:, 4:8, 17:32]).then_inc(s_xg[3], 16)
            # stores for the vector-drained banks (vector can't issue DMAs)
            for cnt, (j, n, half, lo, hi) in enumerate(VBANKS, 1):
                sync.wait_ge(s_act_v, cnt)
                sync.dma_start(o_d[n, :, 450 * half + lo:450 * half + hi],
                               o_sb[:, j, lo:hi]).then_inc(s_o[j], 16)

        @block.scalar
        def _(scalar):
            scalar.dma_start(b_sb[:], b_d[:]).then_inc(s_b, 16)
            scalar.dma_start(w_sb[:, 3:6], w_d[:, 3:6]).then_inc(s_wg[1], 16)
            scalar.dma_start(w_sb[:, 6:9], w_d[:, 6:9]).then_inc(s_wg[2], 16)
            for g, (n0, half) in enumerate(GROUPS):
                for i in (0, 2):
                    j = 4 * (g % 2) + i               # bank & o_sb column
                    n = n0 + i                        # sample
                    scalar.wait_ge(s_mm, 4 * g + i + 1)   # bank accumulated
                    if g == 0 and i == 0:
                        scalar.wait_ge(s_b, 16)       # bias landed
                    if g >= 2:
                        # o_sb column free once its half-0 store drained
                        scalar.wait_ge(s_o[j], 16)
                    nc.scalar.activation(o_sb[:, j], ps[:, j, :PX],
                                         IDENT, bias=b_sb[:]).then_inc(s_act_s, 1)
                    scalar.dma_start(o_d[n, :, 450 * half:450 * half + PX],
                                     o_sb[:, j]).then_inc(s_o[j], 16)
            # tail split: second half of the final bank (7, sample 7, half 1)
            scalar.wait_ge(s_mm, 16)
            scalar.wait_ge(s_o[7], 16)    # col 7's half-0 store drained
            nc.scalar.activation(o_sb[:, 7, PX // 2:PX],
                                 ps[:, 7, PX // 2:PX], IDENT, bias=b_sb[:])
            scalar.dma_start(o_d[7, :, 450 + PX // 2:900],
                             o_sb[:, 7, PX // 2:PX]).then_inc(s_o[7], 16)

        @block.vector
        def _(vector):
            for cnt, (j, n, half, lo, hi) in enumerate(VBANKS, 1):
                g = (cnt - 1) // 2
                vector.wait_ge(s_mm, 4 * g + (j % 4) + 1)   # bank accumulated
                if cnt == 1:
                    vector.wait_ge(s_b, 16)           # bias landed
                if g >= 2:
                    vector.wait_ge(s_o[j], 16)        # column store drained
                nc.vector.tensor_scalar_add(o_sb[:, j, lo:hi], ps[:, j, lo:hi],
                                            b_sb[:]).then_inc(s_act_v, 1)

        @block.tensor
        def _(tensor):
            # Self-loading warmup matmuls on whatever is in SBUF: the PE is
            # busy from kernel entry, so the HAM clock gate opens while the
            # first x DMAs land.  Results go to bank 7, which group 1's
            # start=True matmul resets before any scalar read.
            for _ in range(NWARM):
                nc.tensor.matmul(ps[:, 7, :], w_sb[:, 0], x_sb[:, 0, 0:16, :],
                                 start=True, stop=True, skip_group_check=True)
            for g, (n0, half) in enumerate(GROUPS):
                bank0 = 4 * (g % 2)
                for k in range(KK):
                    p, q = divmod(k, KW)
                    r0 = 15 * half + p
                    ldw = nc.tensor.ldweights(w_sb[:, k])
                    if g == 0 and k in (0, 3, 6):
                        ldw._wait_ge(s_wg[k // 3], 16)   # tap group landed
                    if g >= 1 and k == 0:
                        ldw._wait_ge(s_xg[g], 16)        # x group landed
                    for i in range(4):
                        jb = bank0 + i
                        mm = nc.tensor.matmul(
                            ps[:, jb, :PX],
                            w_sb[:, k],
                            x_sb[:, n0 + i, r0:r0 + 15, q:q + OW],
                            start=(k == 0),
                            stop=(k == KK - 1),
                        )
                        mm.ins.ldweights = False
                        if g == 0 and k == 0:
                            mm._wait_ge(s_x0[i], 16)     # sample landed
                        if g >= 2 and k == 0:
                            # bank drained before we reset it; per-engine
                            # counts make the wait bank-deterministic
                            s = s_act_s if i % 2 == 0 else s_act_v
                            mm._wait_ge(s, 2 * (g - 2) + i // 2 + 1)
                        if k == KK - 1:
                            mm.then_inc(s_mm, 1)

      nc.all_engine_barrier = _orig_barrier

    nc.compile()
    return nc


_NC = None


def _get_nc():
    global _NC
    if _NC is None:
        _NC = _build()
    return _NC


def _in_maps(x, w, bias):
    w_prep = np.ascontiguousarray(
        w.transpose(1, 2, 3, 0).reshape(C, KK, F).astype(np.float16))
    b_prep = np.ascontiguousarray(bias.astype(np.float32).reshape(F, 1))
    maps = []
    for c in range(NCORES):
        xc = np.ascontiguousarray(
            x[c * NPC:(c + 1) * NPC].transpose(1, 0, 2, 3).astype(np.float16))
        maps.append({"x": xc, "w": w_prep, "bias": b_prep})
    return maps


def run(x, w, bias, trace=False, **spmd_kwargs):
    """Run the SPMD kernel; returns (out [N,F,OH,OW], BassKernelResults)."""
    nc = _get_nc()
    res = run_bass_kernel_spmd(nc, _in_maps(x, w, bias), list(range(NCORES)),
                               trace=trace, **spmd_kwargs)
    parts = [res.results[c]["out"].reshape(NPC, F, OH, OW) for c in range(NCORES)]
    return np.concatenate(parts, axis=0), res


def kernel(x, w, bias):
    out, _ = run(np.asarray(x), np.asarray(w), np.asarray(bias))
    return out
